# revision 1
# baseline (speedup 1.0000x reference)
"""Gcs pairwise-distance loss kernel for Trainium2 (Bass/Tile), 8-core SPMD.

Math: with d = pred - truth, dX = d[:, :P], dY = d[:, P:] (B=32, P=1024),
    sumsq_h[i] = sum_{b,j} (v[b,j] - v[b,i])^2
               = S2_h + sum_b (1024*v[b,i]^2 - 2*rs_h[b]*v[b,i])
where rs_h[b] = sum_j v[b,j], S2_h = sum_{b,j} v[b,j]^2.  The loss is
    (sum_i sqrt(sumsq_X[i]) + sum_i sqrt(sumsq_Y[i])) / 64.
This collapses the O(B*P^2) pairwise reduction to O(B*P).

Layout: d [32, 2048] is viewed as [128, 512]; partition p = 4*b + c where
c in {0,1} covers X columns and {2,3} covers Y columns.  Per-partition
free-axis reduces give chunk sums; tiny masked matmuls re-associate the
partition-axis sums; a final Sqrt activation with per-row bias and a
4-element dot produce the scalar.

Schedule notes (from neuron-profile traces):
- sync HWDGE issue is ~0.65us per dma_start, so pred halves go on sync and
  truth halves go on scalar's SWDGE queue to overlap issue; consts ride gpsimd.
- column-split halves let sub/reduce/square run under the h1 transfers.
- the pair-sum matmul runs in bf16 (its weights are exactly -2/0 and cs_d
  only feeds the dot term, ~0.5% of sumsq, so bf16 rounding is invisible);
  fp32 matmuls cost two PE passes.
- ScalarE only ever runs Sqrt so its single ACT table load hides under DMA.

Every core computes the full replicated result (inputs are only 512KB,
far below the ~20us collective all-reduce floor, so replication beats
batch-sharding + AllReduce); core 0's scalar is returned.
"""

import numpy as np

_CACHE = {}


def _build_consts():
    # fp32 [128, 137]:
    #   cols 0:4    mask01[p,m]  = 1 if p%4==m            (lhsT, main matmul)
    #   cols 4:8    maskS[p,m]   = 1/1024 if (p%4)//2==m//2 (lhsT, S2 matmul)
    #   cols 8:136  unused (kept for layout stability)
    #   col  136    q4[p]        = 1/64 for p<4           (rhs, final dot)
    # bf16 [128, 128]: hconst[k,m] = -2 if k//2==m//2     (lhsT, pair sums)
    import ml_dtypes

    c = np.zeros((128, 137), dtype=np.float32)
    p = np.arange(128)
    for m in range(4):
        c[p[p % 4 == m], m] = 1.0
        c[p[(p % 4) // 2 == m // 2], 4 + m] = 1.0 / 1024.0 / 4096.0
    c[0:4, 136] = 1.0 / 64.0
    h = np.zeros((128, 128), dtype=np.float32)
    k = np.arange(128)
    for m in range(128):
        h[k[k // 2 == m // 2], m] = -2.0
    return c, h.astype(ml_dtypes.bfloat16)


def _build_nc():
    import concourse.tile as tile
    from concourse import bacc, mybir

    f32 = mybir.dt.float32
    bf16 = mybir.dt.bfloat16
    nc = bacc.Bacc("TRN2", target_bir_lowering=False, debug=False)
    pred = nc.dram_tensor("pred", [128, 512], f32, kind="ExternalInput").ap()
    truth = nc.dram_tensor("truth", [128, 512], f32, kind="ExternalInput").ap()
    consts = nc.dram_tensor("consts", [128, 137], f32, kind="ExternalInput").ap()
    constsb = nc.dram_tensor("constsb", [128, 128], bf16, kind="ExternalInput").ap()
    out = nc.dram_tensor("out", [1, 1], f32, kind="ExternalOutput").ap()

    H = 256  # column split for DMA/compute overlap

    with tile.TileContext(nc) as tc:
        with (
            tc.tile_pool(name="sb", bufs=1) as sb,
            tc.tile_pool(name="ps", bufs=1, space="PSUM") as ps,
        ):
            tcst = sb.tile([128, 137], f32, tag="tcst")
            nc.gpsimd.dma_start(tcst[:, :], consts)
            tcstb = sb.tile([128, 128], bf16, tag="tcstb")
            nc.gpsimd.dma_start(tcstb[:, :], constsb)
            mask01 = tcst[:, 0:4]
            maskS = tcst[:, 4:8]
            q4 = tcst[0:4, 136:137]

            # pred halves on sync (HWDGE), truth halves on scalar (SWDGE):
            # two issue pipelines instead of four serial 0.65us issues
            # (measured faster than 4-on-sync, which serializes per-queue).
            tp0 = sb.tile([128, H], f32, tag="tp0")
            tt0 = sb.tile([128, H], f32, tag="tt0")
            tp1 = sb.tile([128, H], f32, tag="tp1")
            tt1 = sb.tile([128, H], f32, tag="tt1")
            nc.sync.dma_start(tp0[:, :], pred[:, 0:H])
            nc.scalar.dma_start(tt0[:, :], truth[:, 0:H])
            nc.sync.dma_start(tp1[:, :], pred[:, H:512])
            nc.scalar.dma_start(tt1[:, :], truth[:, H:512])

            td = sb.tile([128, 512], f32, tag="td")
            dsq0 = sb.tile([128, H], f32, tag="dsq0")
            dsq1 = sb.tile([128, H], f32, tag="dsq1")
            acc0 = sb.tile([128, 1], f32, tag="acc0")
            acc1 = sb.tile([128, 1], f32, tag="acc1")
            red0 = sb.tile([128, 1], f32, tag="red0")
            red1 = sb.tile([128, 1], f32, tag="red1")

            # DVE: subs, row-sums, squares (scalar_tensor_tensor + free
            # per-chunk accum; tensor_tensor_reduce crashes TRN2).  ScalarE
            # keeps exactly one activation (Sqrt) so its table loads once,
            # early, hidden under the DMAs.
            nc.vector.tensor_sub(td[:, 0:H], tp0[:, :], tt0[:, :])
            nc.vector.tensor_reduce(
                out=red0[:, :], in_=td[:, 0:H], axis=mybir.AxisListType.X,
                op=mybir.AluOpType.add,
            )
            nc.vector.scalar_tensor_tensor(
                out=dsq0[:, :], in0=td[:, 0:H], scalar=1024.0, in1=td[:, 0:H],
                op0=mybir.AluOpType.mult, op1=mybir.AluOpType.mult,
                accum_out=acc0[:, :],
            )
            nc.vector.tensor_sub(td[:, H:512], tp1[:, :], tt1[:, :])
            nc.vector.tensor_reduce(
                out=red1[:, :], in_=td[:, H:512], axis=mybir.AxisListType.X,
                op=mybir.AluOpType.add,
            )
            nc.vector.scalar_tensor_tensor(
                out=dsq1[:, :], in0=td[:, H:512], scalar=1024.0, in1=td[:, H:512],
                op0=mybir.AluOpType.mult, op1=mybir.AluOpType.mult,
                accum_out=acc1[:, :],
            )
            # cs_d in bf16 feeds only the pair-sum matmul (dot term)
            cs_db = sb.tile([128, 1], bf16, tag="cs_db")
            with tc.high_priority():
                nc.vector.tensor_add(cs_db[:, :], red0[:, :], red1[:, :])
            cs1024 = sb.tile([128, 1], f32, tag="cs1024")
            nc.vector.tensor_add(cs1024[:, :], acc0[:, :], acc1[:, :])

            # hsm2[p] = -2*(cs_d[p] + cs_d[p^1]) — bf16 single-pass matmul
            hconst = tcstb[:, 0:128]
            hsm2 = ps.tile([128, 1], f32, tag="hsm2")
            nc.tensor.matmul(hsm2[:, :], hconst, cs_db[:, :], start=True, stop=True)
            hsm2_sb = sb.tile([128, 1], f32, tag="hsm2_sb")
            nc.vector.tensor_copy(hsm2_sb[:, :], hsm2[:, :])

            # S2 per output row (fp32; feeds only the sqrt bias)
            s2 = ps.tile([4, 1], f32, tag="s2")
            nc.tensor.matmul(s2[:, :], maskS, cs1024[:, :], start=True, stop=True)

            # comb = d*hsm2 + 1024*d^2; PE consumes half 0 while DVE does h1
            main = ps.tile([4, 512], f32, tag="main")
            comb0 = sb.tile([128, H], f32, tag="comb0")
            nc.vector.scalar_tensor_tensor(
                out=comb0[:, :], in0=td[:, 0:H], scalar=hsm2_sb[:, :],
                in1=dsq0[:, :],
                op0=mybir.AluOpType.mult, op1=mybir.AluOpType.add,
            )
            nc.tensor.matmul(main[:, 0:H], mask01, comb0[:, :], start=True, stop=True)
            comb1 = sb.tile([128, H], f32, tag="comb1")
            nc.vector.scalar_tensor_tensor(
                out=comb1[:, :], in0=td[:, H:512], scalar=hsm2_sb[:, :],
                in1=dsq1[:, :],
                op0=mybir.AluOpType.mult, op1=mybir.AluOpType.add,
            )
            nc.tensor.matmul(main[:, H:512], mask01, comb1[:, :], start=True, stop=True)

            bias = sb.tile([4, 1], f32, tag="bias")
            nc.vector.tensor_copy(bias[:, :], s2[:, :])

            # dist = sqrt(main + bias); dsums[m] = sum_j dist[m,j]
            dist = sb.tile([4, 512], f32, tag="dist")
            dsums = sb.tile([4, 1], f32, tag="dsums")
            # scale=2^-12 folds the /64 into the sqrt: sqrt(x/4096)=sqrt(x)/64
            nc.scalar.activation(
                dist[:, :], main[:, :], mybir.ActivationFunctionType.Sqrt,
                bias=bias[:, :], scale=1.0 / 4096.0, accum_out=dsums[:, :],
            )

            # total = sum_m dsums[m]  (4-partition sum on gpsimd)
            out_sb = sb.tile([1, 1], f32, tag="out_sb")
            nc.gpsimd.tensor_reduce(
                out=out_sb[:, :], in_=dsums[:, :], axis=mybir.AxisListType.C,
                op=mybir.AluOpType.add,
            )
            nc.sync.dma_start(out, out_sb[:, :])

    nc.compile()
    return nc


def _get():
    if "nc" not in _CACHE:
        _CACHE["nc"] = _build_nc()
        _CACHE["consts"], _CACHE["constsb"] = _build_consts()
    return _CACHE["nc"], _CACHE["consts"]


def _in_map(pred, truth):
    nc, consts = _get()
    p = np.ascontiguousarray(np.asarray(pred, dtype=np.float32)).reshape(128, 512)
    t = np.ascontiguousarray(np.asarray(truth, dtype=np.float32)).reshape(128, 512)
    return nc, {"pred": p, "truth": t, "consts": consts,
                "constsb": _CACHE["constsb"]}


def kernel(pred, truth) -> np.ndarray:
    from concourse.bass_utils import run_bass_kernel_spmd

    nc, in_map = _in_map(pred, truth)
    res = run_bass_kernel_spmd(
        nc, [dict(in_map) for _ in range(8)], core_ids=list(range(8))
    )
    return res.results[0]["out"].reshape(()).astype(np.float32)



# revision 8
# speedup vs baseline: 1.1201x; 1.1201x over previous
"""Gcs pairwise-distance loss kernel for Trainium2 (Bass/Tile), 8-core SPMD.

Math: with d = pred - truth, dX = d[:, :P], dY = d[:, P:] (B=32, P=1024),
    sumsq_h[i] = sum_{b,j} (v[b,j] - v[b,i])^2
               = S2_h + sum_b (1024*v[b,i]^2 - 2*rs_h[b]*v[b,i])
where rs_h[b] = sum_j v[b,j], S2_h = sum_{b,j} v[b,j]^2.  The loss is
    (sum_i sqrt(sumsq_X[i]) + sum_i sqrt(sumsq_Y[i])) / 64.
This collapses the O(B*P^2) pairwise reduction to O(B*P).

Layout: d [32, 2048] is viewed as [128, 512]; partition p = 4*b + c where
c in {0,1} covers X columns and {2,3} covers Y columns.

v4 design notes (driven by NTFF traces of v1-v3):
- exec_time is measured first-MEMSET -> last instruction; a fixed ~7us
  NEFF postamble (per-engine semaphore-file zeroing) plus ~2us of
  out-DMA receipt + end barriers bound the floor.
- everything runs in bf16 (tolerance 2e-2; measured error ~1e-5 in v2).
- v3 lesson: PSUM accumulation groups must not interleave - a start
  from another group between one group's start and stop corrupts the
  accumulation (s2 lost its first term).  v4 makes EVERY matmul
  self-contained (start=stop=True): the dot term is folded into
    comb[p,j] = td[p,j]*hsm2[p] + dsq[p,j]   (one DVE STT per half)
  with hsm2 = pair-matmul(hconst2, r), hconst2 = -2/1024 so that
    main[m,j] = sum_p (1024*mask01)[p,m]*comb[p,j] = sumsq - S2.
  Both main halves are flags-3 matmuls into disjoint regions of one
  PSUM bank; the two S2 partials land in adjacent PSUM columns and
  merge with a tiny tensor_reduce.
- three DMA issue queues: pred halves on sync (HWDGE), truth h0 +
  consts on scalar (HWDGE), truth h1 on gpsimd (SWDGE): the 64KB
  transfers are latency-bound, so the last ISSUE end is what matters.
- per-half row sums on DVE (bf16 tensor_reduce) fill the gap while
  h1 is in flight; per-half ScalarE Squares produce dsq + ssb accums
  concurrently with the DVE subs/reduces.
- dsums accumulates in bf16 so the final 4-row dot is a single-pass
  bf16 matmul (q4 ones weights); gpsimd never needs a library load.
- a dummy Sqrt right after the DMA issues pulls the single ACT table
  load (set 3 = sqrt+square) into the DMA-flight window.

Every core computes the full replicated result (inputs are only 256KB
in bf16, far below the ~20us collective all-reduce floor, so replication
beats batch-sharding + AllReduce); core 0's scalar is returned.
"""

import numpy as np

_CACHE = {}

H = 256  # column split of the 512 free columns for DMA/compute pipelining


def _build_consts():
    # bf16 [128, 137]:
    #   cols 0:128    hconst2[p,m]  = -2^-9 if p//2==m//2  (lhsT, pair-sum matmul)
    #   cols 128:132  maskS[p,m]    = 2^-12 if (p%4)//2==m//2  (lhsT, S2 matmuls)
    #   cols 132:136  mask1024[p,m] = 1024 if p%4==m       (lhsT, comb matmuls)
    #   col  136      q4[p]         = 1 if p<4             (lhsT, final dot)
    import ml_dtypes

    c = np.zeros((128, 137), dtype=np.float32)
    p = np.arange(128)
    for m in range(128):
        c[p[p // 2 == m // 2], m] = -(2.0**-9)
    for m in range(4):
        c[p[(p % 4) // 2 == m // 2], 128 + m] = 2.0**-12
        c[p[p % 4 == m], 132 + m] = 1024.0
    c[0:4, 136] = 1.0
    return c.astype(ml_dtypes.bfloat16)


def _build_nc():
    import concourse.tile as tile
    from concourse import bacc, mybir

    f32 = mybir.dt.float32
    bf16 = mybir.dt.bfloat16
    Sq = mybir.ActivationFunctionType.Square
    nc = bacc.Bacc("TRN2", target_bir_lowering=False, debug=False)
    pred = nc.dram_tensor("pred", [128, 512], bf16, kind="ExternalInput").ap()
    truth = nc.dram_tensor("truth", [128, 512], bf16, kind="ExternalInput").ap()
    consts = nc.dram_tensor("consts", [128, 137], bf16, kind="ExternalInput").ap()
    out = nc.dram_tensor("out", [1, 1], f32, kind="ExternalOutput").ap()

    with tile.TileContext(nc) as tc:
        with (
            tc.tile_pool(name="sb", bufs=1) as sb,
            tc.tile_pool(name="ps", bufs=1, space="PSUM") as ps,
        ):
            tp0 = sb.tile([128, H], bf16, tag="tp0")
            tt0 = sb.tile([128, H], bf16, tag="tt0")
            tp1 = sb.tile([128, H], bf16, tag="tp1")
            tt1 = sb.tile([128, H], bf16, tag="tt1")
            tcst = sb.tile([128, 137], bf16, tag="tcst")
            nc.sync.dma_start(tp0[:, :], pred[:, 0:H])
            nc.scalar.dma_start(tt0[:, :], truth[:, 0:H])
            nc.gpsimd.dma_start(tt1[:, :], truth[:, H:512])
            nc.sync.dma_start(tp1[:, :], pred[:, H:512])
            nc.scalar.dma_start(tcst[:, :], consts)
            hconst2 = tcst[:, 0:128]
            maskS = tcst[:, 128:132]
            mask1024 = tcst[:, 132:136]
            q4 = tcst[0:4, 136:137]

            # Dummy Sqrt on a bass preamble const: pulls the single ACT
            # table load (set 3: sqrt+square+copy) into the DMA-flight
            # window instead of right before the final Sqrt.
            dummy = sb.tile([128, 1], f32, tag="dummy")
            nc.scalar.activation(
                dummy[:, :],
                nc.const_aps.aps[(f32, 1.0)],
                mybir.ActivationFunctionType.Sqrt,
            )

            td = sb.tile([128, 512], bf16, tag="td")
            red0 = sb.tile([128, 1], bf16, tag="red0")
            red1 = sb.tile([128, 1], bf16, tag="red1")
            dsq = sb.tile([128, 512], bf16, tag="dsq")
            ssb0 = sb.tile([128, 1], bf16, tag="ssb0")
            ssb1 = sb.tile([128, 1], bf16, tag="ssb1")

            # h0: sub + row sums (DVE) and squares + ssb accum (ACT)
            # run in the gap while h1 is still in flight.
            nc.vector.tensor_sub(td[:, 0:H], tp0[:, :], tt0[:, :])
            with nc.allow_low_precision("row sums feed the ~0.5% dot term"):
                nc.vector.tensor_reduce(
                    out=red0[:, :], in_=td[:, 0:H], axis=mybir.AxisListType.X,
                    op=mybir.AluOpType.add,
                )
            with nc.allow_low_precision("ssb: S2 sums in bf16, ~0.4% noise"):
                nc.scalar.activation(
                    dsq[:, 0:H], td[:, 0:H], Sq, accum_out=ssb0[:, :],
                )
            # h1
            nc.vector.tensor_sub(td[:, H:512], tp1[:, :], tt1[:, :])
            with nc.allow_low_precision("row sums feed the ~0.5% dot term"):
                nc.vector.tensor_reduce(
                    out=red1[:, :], in_=td[:, H:512], axis=mybir.AxisListType.X,
                    op=mybir.AluOpType.add,
                )
            with nc.allow_low_precision("ssb: S2 sums in bf16, ~0.4% noise"):
                nc.scalar.activation(
                    dsq[:, H:512], td[:, H:512], Sq, accum_out=ssb1[:, :],
                )

            # r = red0 + red1; hsm2[p] = -2^-9*(r[p]+r[p^1]) = -2*rs/1024
            r_bf = sb.tile([128, 1], bf16, tag="r_bf")
            nc.vector.tensor_add(r_bf[:, :], red0[:, :], red1[:, :])
            hsm2 = ps.tile([128, 1], f32, tag="hsm2")
            nc.tensor.matmul(hsm2[:, :], hconst2, r_bf[:, :], start=True, stop=True)
            hsm2_sb = sb.tile([128, 1], f32, tag="hsm2_sb")
            nc.vector.tensor_copy(hsm2_sb[:, :], hsm2[:, :])

            # s2 partials: maskS * ssb_h into adjacent PSUM columns
            # (self-contained matmuls), merged below by a tiny reduce.
            s2p = ps.tile([4, 2], f32, tag="s2p")
            nc.tensor.matmul(s2p[:, 0:1], maskS, ssb0[:, :], start=True, stop=True)
            nc.tensor.matmul(s2p[:, 1:2], maskS, ssb1[:, :], start=True, stop=True)

            # comb = td*hsm2 + dsq; main = 1024*sum_b comb = sumsq - S2
            comb0 = sb.tile([128, H], bf16, tag="comb0")
            comb1 = sb.tile([128, H], bf16, tag="comb1")
            main = ps.tile([4, 512], f32, tag="main")
            nc.vector.scalar_tensor_tensor(
                out=comb0[:, :], in0=td[:, 0:H], scalar=hsm2_sb[:, :],
                in1=dsq[:, 0:H],
                op0=mybir.AluOpType.mult, op1=mybir.AluOpType.add,
            )
            nc.tensor.matmul(main[:, 0:H], mask1024, comb0[:, :], start=True, stop=True)
            nc.vector.scalar_tensor_tensor(
                out=comb1[:, :], in0=td[:, H:512], scalar=hsm2_sb[:, :],
                in1=dsq[:, H:512],
                op0=mybir.AluOpType.mult, op1=mybir.AluOpType.add,
            )
            nc.tensor.matmul(main[:, H:512], mask1024, comb1[:, :], start=True, stop=True)

            # bias[m] = 2^-12 * S2_half = s2p[:,0] + s2p[:,1]
            bias = sb.tile([4, 1], f32, tag="bias")
            nc.vector.tensor_reduce(
                out=bias[:, :], in_=s2p[:, 0:2], axis=mybir.AxisListType.X,
                op=mybir.AluOpType.add,
            )

            # dist = sqrt(main*2^-12 + bias) = sqrt(sumsq)/64; dsums[m]
            # accumulates in bf16 so the final dot is single-pass bf16.
            dist = sb.tile([4, 512], bf16, tag="dist")
            dsums = sb.tile([4, 1], bf16, tag="dsums")
            with nc.allow_low_precision("dsums ~2900, bf16 rounding ~0.03%"):
                nc.scalar.activation(
                    dist[:, :], main[:, :], mybir.ActivationFunctionType.Sqrt,
                    bias=bias[:, :], scale=2.0**-12, accum_out=dsums[:, :],
                )

            # total = sum_m dsums[m]  (tiny bf16 PE dot)
            total = ps.tile([1, 1], f32, tag="total")
            nc.tensor.matmul(total[:, :], q4, dsums[:, :], start=True, stop=True)
            out_sb = sb.tile([1, 1], f32, tag="out_sb")
            nc.vector.tensor_copy(out_sb[:, :], total[:, :])
            nc.sync.dma_start(out, out_sb[:, :])

    nc.compile()
    return nc


def _get():
    if "nc" not in _CACHE:
        _CACHE["nc"] = _build_nc()
        _CACHE["consts"] = _build_consts()
    return _CACHE["nc"], _CACHE["consts"]


def _in_map(pred, truth):
    import ml_dtypes

    nc, consts = _get()
    p = np.ascontiguousarray(
        np.asarray(pred, dtype=np.float32).reshape(128, 512).astype(ml_dtypes.bfloat16)
    )
    t = np.ascontiguousarray(
        np.asarray(truth, dtype=np.float32).reshape(128, 512).astype(ml_dtypes.bfloat16)
    )
    return nc, {"pred": p, "truth": t, "consts": consts}


def kernel(pred, truth) -> np.ndarray:
    from concourse.bass_utils import run_bass_kernel_spmd

    nc, in_map = _in_map(pred, truth)
    res = run_bass_kernel_spmd(
        nc, [dict(in_map) for _ in range(8)], core_ids=list(range(8))
    )
    return res.results[0]["out"].reshape(()).astype(np.float32)


# revision 9
# speedup vs baseline: 1.1508x; 1.0274x over previous
"""Gcs pairwise-distance loss kernel for Trainium2 (Bass/Tile), 8-core SPMD.

Math: with d = pred - truth, dX = d[:, :P], dY = d[:, P:] (B=32, P=1024),
    sumsq_h[i] = sum_{b,j} (v[b,j] - v[b,i])^2
               = S2_h + sum_b (1024*v[b,i]^2 - 2*rs_h[b]*v[b,i])
where rs_h[b] = sum_j v[b,j], S2_h = sum_{b,j} v[b,j]^2.  The loss is
    (sum_i sqrt(sumsq_X[i]) + sum_i sqrt(sumsq_Y[i])) / 64.
This collapses the O(B*P^2) pairwise reduction to O(B*P).

Layout: d [32, 2048] is viewed as [128, 512]; partition p = 4*b + c where
c in {0,1} covers X columns and {2,3} covers Y columns.

v5 design notes (driven by NTFF traces of v1-v4):
- exec_time is measured first-MEMSET -> last instruction; a fixed ~7us
  NEFF postamble (per-engine semaphore-file zeroing) plus ~2.4us of
  out-DMA receipt + end barriers bound the floor.
- everything runs in bf16 (tolerance 2e-2; measured error ~9e-4).
- PSUM accumulation rule learned in v3: start=True clears the whole
  target BANK's has_written bits, so an accumulation group may not
  have another start TO THE SAME BANK between its start and stop;
  starts to OTHER banks interleave fine.  v5 exploits that:
    hsm2 bank:  pairA(start, red0) ... pairB(stop, red1)
    main bank:  mm1a(start, dsq h0), mm1b(-, dsq h1),
                mm2a(-, W2*td h0), mm2b(stop, W2*td h1)
  (mm1b hits has_written-cleared region b, which writes fresh; the
  mm2 matmuls then accumulate - one group, no same-bank restarts.)
- W2[p,m] = mask01[p,m]*hsm2[p] with the per-partition scalar read
  straight from PSUM (fp32), so the dot term costs one tiny DVE op
  plus PE work that overlaps the DVE/ACT chain.
- per-half row sums on DVE (bf16 tensor_reduce) fill the gap while
  h1 is in flight; per-half ScalarE Squares produce dsq + ssb accums
  concurrently; ssb0+ssb1 merge on DVE feeds one maskS matmul.
- three DMA issue queues: pred halves on sync (HWDGE), truth h0 +
  consts on scalar (HWDGE), truth h1 on gpsimd (SWDGE): the 64KB
  transfers are latency-bound, so the last ISSUE end is what matters.
- dsums accumulates in bf16 so the final 4-row dot is a single-pass
  bf16 matmul; gpsimd never needs a custom-op library load.
- a dummy Sqrt right after the DMA issues pulls the single ACT table
  load (set 3 = sqrt+square) into the DMA-flight window.

Every core computes the full replicated result (inputs are only 256KB
in bf16, far below the ~20us collective all-reduce floor, so replication
beats batch-sharding + AllReduce); core 0's scalar is returned.
"""

import numpy as np

_CACHE = {}

H = 256  # column split of the 512 free columns for DMA/compute pipelining


def _build_consts():
    # bf16 [128, 141]:
    #   cols 0:128    hconst[p,m]   = -2 if p//2==m//2    (lhsT, pair-sum matmuls)
    #   cols 128:132  mask01[p,m]   = 1 if p%4==m         (base for W2 weights)
    #   cols 132:136  maskS[p,m]    = 2^-12 if (p%4)//2==m//2  (lhsT, S2 matmul)
    #   cols 136:140  mask1024[p,m] = 1024 if p%4==m      (lhsT, d^2 matmuls)
    #   col  140      q4[p]         = 1 if p<4            (lhsT, final dot)
    import ml_dtypes

    c = np.zeros((128, 141), dtype=np.float32)
    p = np.arange(128)
    for m in range(128):
        c[p[p // 2 == m // 2], m] = -2.0
    for m in range(4):
        c[p[p % 4 == m], 128 + m] = 1.0
        c[p[(p % 4) // 2 == m // 2], 132 + m] = 2.0**-12
        c[p[p % 4 == m], 136 + m] = 1024.0
    c[0:4, 140] = 1.0
    return c.astype(ml_dtypes.bfloat16)


def _build_nc():
    import concourse.tile as tile
    from concourse import bacc, mybir

    f32 = mybir.dt.float32
    bf16 = mybir.dt.bfloat16
    Sq = mybir.ActivationFunctionType.Square
    nc = bacc.Bacc("TRN2", target_bir_lowering=False, debug=False)
    pred = nc.dram_tensor("pred", [128, 512], bf16, kind="ExternalInput").ap()
    truth = nc.dram_tensor("truth", [128, 512], bf16, kind="ExternalInput").ap()
    consts = nc.dram_tensor("consts", [128, 141], bf16, kind="ExternalInput").ap()
    out = nc.dram_tensor("out", [1, 1], f32, kind="ExternalOutput").ap()

    with tile.TileContext(nc) as tc:
        with (
            tc.tile_pool(name="sb", bufs=1) as sb,
            tc.tile_pool(name="ps", bufs=1, space="PSUM") as ps,
        ):
            tp0 = sb.tile([128, H], bf16, tag="tp0")
            tt0 = sb.tile([128, H], bf16, tag="tt0")
            tp1 = sb.tile([128, H], bf16, tag="tp1")
            tt1 = sb.tile([128, H], bf16, tag="tt1")
            tcst = sb.tile([128, 141], bf16, tag="tcst")
            nc.sync.dma_start(tp0[:, :], pred[:, 0:H])
            nc.scalar.dma_start(tt0[:, :], truth[:, 0:H])
            nc.gpsimd.dma_start(tt1[:, :], truth[:, H:512])
            nc.sync.dma_start(tp1[:, :], pred[:, H:512])
            nc.scalar.dma_start(tcst[:, :], consts)
            hconst = tcst[:, 0:128]
            mask01 = tcst[:, 128:132]
            maskS = tcst[:, 132:136]
            mask1024 = tcst[:, 136:140]
            q4 = tcst[0:4, 140:141]

            # Dummy Sqrt on a bass preamble const: pulls the single ACT
            # table load (set 3: sqrt+square+copy) into the DMA-flight
            # window instead of right before the final Sqrt.
            dummy = sb.tile([128, 1], f32, tag="dummy")
            nc.scalar.activation(
                dummy[:, :],
                nc.const_aps.aps[(f32, 1.0)],
                mybir.ActivationFunctionType.Sqrt,
            )

            td = sb.tile([128, 512], bf16, tag="td")
            red0 = sb.tile([128, 1], bf16, tag="red0")
            red1 = sb.tile([128, 1], bf16, tag="red1")
            dsq = sb.tile([128, 512], bf16, tag="dsq")
            ssb0 = sb.tile([128, 1], bf16, tag="ssb0")
            ssb1 = sb.tile([128, 1], bf16, tag="ssb1")

            # h0: sub + row sums (DVE) and squares + ssb accum (ACT)
            # run in the gap while h1 is still in flight.
            nc.vector.tensor_sub(td[:, 0:H], tp0[:, :], tt0[:, :])
            with nc.allow_low_precision("row sums feed the ~0.5% dot term"):
                nc.vector.tensor_reduce(
                    out=red0[:, :], in_=td[:, 0:H], axis=mybir.AxisListType.X,
                    op=mybir.AluOpType.add,
                )
            with nc.allow_low_precision("ssb: S2 sums in bf16, ~0.4% noise"):
                nc.scalar.activation(
                    dsq[:, 0:H], td[:, 0:H], Sq, accum_out=ssb0[:, :],
                )
            # h1
            nc.vector.tensor_sub(td[:, H:512], tp1[:, :], tt1[:, :])
            with nc.allow_low_precision("row sums feed the ~0.5% dot term"):
                nc.vector.tensor_reduce(
                    out=red1[:, :], in_=td[:, H:512], axis=mybir.AxisListType.X,
                    op=mybir.AluOpType.add,
                )
            with nc.allow_low_precision("ssb: S2 sums in bf16, ~0.4% noise"):
                nc.scalar.activation(
                    dsq[:, H:512], td[:, H:512], Sq, accum_out=ssb1[:, :],
                )

            # hsm2[p] = -2*(r[p]+r[p^1]): accumulate red0 then red1 in the
            # hsm2 bank; main bank: one group, one start (mm1a), regions
            # written fresh after the bank-wide has_written clear, then
            # the two W2 matmuls accumulate the dot term on top.
            hsm2 = ps.tile([128, 1], f32, tag="hsm2")
            main = ps.tile([4, 512], f32, tag="main")
            s2 = ps.tile([4, 1], f32, tag="s2")
            nc.tensor.matmul(hsm2[:, :], hconst, red0[:, :], start=True, stop=False)
            nc.tensor.matmul(main[:, 0:H], mask1024, dsq[:, 0:H], start=True, stop=False)
            nc.tensor.matmul(hsm2[:, :], hconst, red1[:, :], start=False, stop=True)
            nc.tensor.matmul(
                main[:, H:512], mask1024, dsq[:, H:512], start=False, stop=False,
                skip_group_check=True,
            )

            # W2[p,m] = mask01[p,m] * hsm2[p], scalar read from PSUM
            w2 = sb.tile([128, 4], bf16, tag="w2")
            nc.vector.tensor_scalar(
                w2[:, :], mask01, hsm2[:, :], None, mybir.AluOpType.mult,
            )
            # ssb = ssb0 + ssb1 feeds one maskS matmul
            ssb = sb.tile([128, 1], bf16, tag="ssb")
            nc.vector.tensor_add(ssb[:, :], ssb0[:, :], ssb1[:, :])

            nc.tensor.matmul(
                main[:, 0:H], w2[:, :], td[:, 0:H], start=False, stop=False,
                skip_group_check=True,
            )
            nc.tensor.matmul(s2[:, :], maskS, ssb[:, :], start=True, stop=True)
            nc.tensor.matmul(
                main[:, H:512], w2[:, :], td[:, H:512], start=False, stop=True,
                skip_group_check=True,
            )

            bias = sb.tile([4, 1], f32, tag="bias")
            nc.vector.tensor_copy(bias[:, :], s2[:, :])

            # dist = sqrt(main*2^-12 + bias) = sqrt(sumsq)/64; dsums[m]
            # accumulates in bf16 so the final dot is single-pass bf16.
            dist = sb.tile([4, 512], bf16, tag="dist")
            dsums = sb.tile([4, 1], bf16, tag="dsums")
            with nc.allow_low_precision("dsums ~2900, bf16 rounding ~0.03%"):
                nc.scalar.activation(
                    dist[:, :], main[:, :], mybir.ActivationFunctionType.Sqrt,
                    bias=bias[:, :], scale=2.0**-12, accum_out=dsums[:, :],
                )

            # total = sum_m dsums[m]  (tiny bf16 PE dot)
            total = ps.tile([1, 1], f32, tag="total")
            nc.tensor.matmul(total[:, :], q4, dsums[:, :], start=True, stop=True)
            out_sb = sb.tile([1, 1], f32, tag="out_sb")
            nc.vector.tensor_copy(out_sb[:, :], total[:, :])
            nc.sync.dma_start(out, out_sb[:, :])

    nc.compile()
    return nc


def _get():
    if "nc" not in _CACHE:
        _CACHE["nc"] = _build_nc()
        _CACHE["consts"] = _build_consts()
    return _CACHE["nc"], _CACHE["consts"]


def _in_map(pred, truth):
    import ml_dtypes

    nc, consts = _get()
    p = np.ascontiguousarray(
        np.asarray(pred, dtype=np.float32).reshape(128, 512).astype(ml_dtypes.bfloat16)
    )
    t = np.ascontiguousarray(
        np.asarray(truth, dtype=np.float32).reshape(128, 512).astype(ml_dtypes.bfloat16)
    )
    return nc, {"pred": p, "truth": t, "consts": consts}


def kernel(pred, truth) -> np.ndarray:
    from concourse.bass_utils import run_bass_kernel_spmd

    nc, in_map = _in_map(pred, truth)
    res = run_bass_kernel_spmd(
        nc, [dict(in_map) for _ in range(8)], core_ids=list(range(8))
    )
    return res.results[0]["out"].reshape(()).astype(np.float32)


# revision 14
# speedup vs baseline: 1.1681x; 1.0150x over previous
"""Gcs pairwise-distance loss kernel for Trainium2 (Bass/Tile), 8-core SPMD.

Math: with d = pred - truth, dX = d[:, :P], dY = d[:, P:] (B=32, P=1024),
    sumsq_h[i] = sum_{b,j} (v[b,j] - v[b,i])^2
               = S2_h + sum_b (1024*v[b,i]^2 - 2*rs_h[b]*v[b,i])
where rs_h[b] = sum_j v[b,j], S2_h = sum_{b,j} v[b,j]^2.  The loss is
    (sum_i sqrt(sumsq_X[i]) + sum_i sqrt(sumsq_Y[i])) / 64.
This collapses the O(B*P^2) pairwise reduction to O(B*P).

Layout: d [32, 2048] is viewed as [128, 512]; partition p = 4*b + c where
c in {0,1} covers X columns and {2,3} covers Y columns.

v5 design notes (driven by NTFF traces of v1-v4):
- exec_time is measured first-MEMSET -> last instruction; a fixed ~7us
  NEFF postamble (per-engine semaphore-file zeroing) plus ~2.4us of
  out-DMA receipt + end barriers bound the floor.
- everything runs in bf16 (tolerance 2e-2; measured error ~9e-4).
- PSUM accumulation rule learned in v3: start=True clears the whole
  target BANK's has_written bits, so an accumulation group may not
  have another start TO THE SAME BANK between its start and stop;
  starts to OTHER banks interleave fine.  v5 exploits that:
    hsm2 bank:  pairA(start, red0) ... pairB(stop, red1)
    main bank:  mm1a(start, dsq h0), mm1b(-, dsq h1),
                mm2a(-, W2*td h0), mm2b(stop, W2*td h1)
  (mm1b hits has_written-cleared region b, which writes fresh; the
  mm2 matmuls then accumulate - one group, no same-bank restarts.)
- W2[p,m] = mask01[p,m]*hsm2[p] with the per-partition scalar read
  straight from PSUM (fp32), so the dot term costs one tiny DVE op
  plus PE work that overlaps the DVE/ACT chain.
- per-half row sums on DVE (bf16 tensor_reduce) fill the gap while
  h1 is in flight; per-half ScalarE Squares produce dsq + ssb accums
  concurrently; ssb0+ssb1 merge on DVE feeds one maskS matmul.
- three DMA issue queues: pred halves on sync (HWDGE), truth halves on
  scalar (HWDGE), consts on gpsimd (SWDGE; only needed by ~4.5us): the
  64KB transfers are latency-bound (~2.1us after issue end), so what
  matters is issuing truth h1 as early as possible on a HWDGE queue.
- dsums accumulates in bf16 so the final 4-row dot is a single-pass
  bf16 matmul; gpsimd never needs a custom-op library load.
- a dummy Sqrt right after the DMA issues forces ONE set-3 table load
  (sqrt+square) early, hidden under the DMA flight; without it the
  table pass reloads set 3 right before the final Sqrt.

Every core computes the full replicated result (inputs are only 256KB
in bf16, far below the ~20us collective all-reduce floor, so replication
beats batch-sharding + AllReduce); core 0's scalar is returned.
"""

import numpy as np

_CACHE = {}

H = 256  # column split of the 512 free columns for DMA/compute pipelining


def _build_consts():
    # bf16 [128, 141]:
    #   cols 0:128    hconst[p,m]   = -2 if p//2==m//2    (lhsT, pair-sum matmuls)
    #   cols 128:132  mask01[p,m]   = 1 if p%4==m         (base for W2 weights)
    #   cols 132:136  maskS[p,m]    = 2^-12 if (p%4)//2==m//2  (lhsT, S2 matmul)
    #   cols 136:140  mask1024[p,m] = 1024 if p%4==m      (lhsT, d^2 matmuls)
    #   col  140      q4[p]         = 1 if p<4            (lhsT, final dot)
    import ml_dtypes

    c = np.zeros((128, 141), dtype=np.float32)
    p = np.arange(128)
    for m in range(128):
        c[p[p // 2 == m // 2], m] = -2.0
    for m in range(4):
        c[p[p % 4 == m], 128 + m] = 1.0
        c[p[(p % 4) // 2 == m // 2], 132 + m] = 2.0**-12
        c[p[p % 4 == m], 136 + m] = 1024.0
    c[0:4, 140] = 1.0
    return c.astype(ml_dtypes.bfloat16)


def _build_nc():
    import concourse.tile as tile
    from concourse import bacc, mybir

    f32 = mybir.dt.float32
    bf16 = mybir.dt.bfloat16
    Sq = mybir.ActivationFunctionType.Square
    nc = bacc.Bacc("TRN2", target_bir_lowering=False, debug=False)
    pred = nc.dram_tensor("pred", [128, 512], bf16, kind="ExternalInput").ap()
    truth = nc.dram_tensor("truth", [128, 512], bf16, kind="ExternalInput").ap()
    consts = nc.dram_tensor("consts", [128, 141], bf16, kind="ExternalInput").ap()
    out = nc.dram_tensor("out", [1, 1], f32, kind="ExternalOutput").ap()

    with tile.TileContext(nc) as tc:
        with (
            tc.tile_pool(name="sb", bufs=1) as sb,
            tc.tile_pool(name="ps", bufs=1, space="PSUM") as ps,
        ):
            tp0 = sb.tile([128, H], bf16, tag="tp0")
            tt0 = sb.tile([128, H], bf16, tag="tt0")
            tp1 = sb.tile([128, H], bf16, tag="tp1")
            tt1 = sb.tile([128, H], bf16, tag="tt1")
            tcst = sb.tile([128, 141], bf16, tag="tcst")
            nc.sync.dma_start(tp0[:, :], pred[:, 0:H])
            nc.scalar.dma_start(tt0[:, :], truth[:, 0:H])
            nc.gpsimd.dma_start(tcst[:, :], consts)
            nc.sync.dma_start(tp1[:, :], pred[:, H:512])
            nc.scalar.dma_start(tt1[:, :], truth[:, H:512])
            hconst = tcst[:, 0:128]
            mask01 = tcst[:, 128:132]
            maskS = tcst[:, 132:136]
            mask1024 = tcst[:, 136:140]
            q4 = tcst[0:4, 140:141]

            # Dummy Sqrt on a bass preamble const: forces the ACT table
            # pass to load set 3 (sqrt+square) early, hidden under the
            # DMA flight, and to keep it for the Squares AND final Sqrt.
            # Without it the pass picks set 0 for the Squares and reloads
            # set 3 right before the final Sqrt (1.28us on the critical
            # path, gated behind the bias wait).
            dummy = sb.tile([128, 1], f32, tag="dummy")
            nc.scalar.activation(
                dummy[:, :],
                nc.const_aps.aps[(f32, 1.0)],
                mybir.ActivationFunctionType.Sqrt,
            )

            td = sb.tile([128, 512], bf16, tag="td")
            red0 = sb.tile([128, 1], bf16, tag="red0")
            red1 = sb.tile([128, 1], bf16, tag="red1")
            dsq = sb.tile([128, 512], bf16, tag="dsq")
            ssb0 = sb.tile([128, 1], bf16, tag="ssb0")
            ssb1 = sb.tile([128, 1], bf16, tag="ssb1")

            # h0: sub + row sums (DVE) and squares + ssb accum (ACT)
            # run in the gap while h1 is still in flight.
            nc.vector.tensor_sub(td[:, 0:H], tp0[:, :], tt0[:, :])
            with nc.allow_low_precision("row sums feed the ~0.5% dot term"):
                nc.vector.tensor_reduce(
                    out=red0[:, :], in_=td[:, 0:H], axis=mybir.AxisListType.X,
                    op=mybir.AluOpType.add,
                )
            with nc.allow_low_precision("ssb: S2 sums in bf16, ~0.4% noise"):
                nc.scalar.activation(
                    dsq[:, 0:H], td[:, 0:H], Sq, accum_out=ssb0[:, :],
                )
            # h1
            nc.vector.tensor_sub(td[:, H:512], tp1[:, :], tt1[:, :])
            with nc.allow_low_precision("row sums feed the ~0.5% dot term"):
                nc.vector.tensor_reduce(
                    out=red1[:, :], in_=td[:, H:512], axis=mybir.AxisListType.X,
                    op=mybir.AluOpType.add,
                )
            with nc.allow_low_precision("ssb: S2 sums in bf16, ~0.4% noise"):
                nc.scalar.activation(
                    dsq[:, H:512], td[:, H:512], Sq, accum_out=ssb1[:, :],
                )

            # hsm2[p] = -2*(r[p]+r[p^1]): accumulate red0 then red1 in the
            # hsm2 bank; main bank: one group, one start (mm1a), regions
            # written fresh after the bank-wide has_written clear, then
            # the two W2 matmuls accumulate the dot term on top.
            hsm2 = ps.tile([128, 1], f32, tag="hsm2")
            main = ps.tile([4, 512], f32, tag="main")
            s2 = ps.tile([4, 1], f32, tag="s2")
            nc.tensor.matmul(hsm2[:, :], hconst, red0[:, :], start=True, stop=False)
            nc.tensor.matmul(main[:, 0:H], mask1024, dsq[:, 0:H], start=True, stop=False)
            nc.tensor.matmul(hsm2[:, :], hconst, red1[:, :], start=False, stop=True)
            nc.tensor.matmul(
                main[:, H:512], mask1024, dsq[:, H:512], start=False, stop=False,
                skip_group_check=True,
            )

            # W2[p,m] = mask01[p,m] * hsm2[p], scalar read from PSUM
            w2 = sb.tile([128, 4], bf16, tag="w2")
            nc.vector.tensor_scalar(
                w2[:, :], mask01, hsm2[:, :], None, mybir.AluOpType.mult,
            )
            # ssb = ssb0 + ssb1 feeds one maskS matmul
            ssb = sb.tile([128, 1], bf16, tag="ssb")
            nc.vector.tensor_add(ssb[:, :], ssb0[:, :], ssb1[:, :])

            nc.tensor.matmul(
                main[:, 0:H], w2[:, :], td[:, 0:H], start=False, stop=False,
                skip_group_check=True,
            )
            nc.tensor.matmul(s2[:, :], maskS, ssb[:, :], start=True, stop=True)
            nc.tensor.matmul(
                main[:, H:512], w2[:, :], td[:, H:512], start=False, stop=True,
                skip_group_check=True,
            )

            bias = sb.tile([4, 1], f32, tag="bias")
            nc.vector.tensor_copy(bias[:, :], s2[:, :])

            # dist = sqrt(main*2^-12 + bias) = sqrt(sumsq)/64; dsums[m]
            # accumulates in bf16 so the final dot is single-pass bf16.
            dist = sb.tile([4, 512], bf16, tag="dist")
            dsums = sb.tile([4, 1], bf16, tag="dsums")
            with nc.allow_low_precision("dsums ~2900, bf16 rounding ~0.03%"):
                nc.scalar.activation(
                    dist[:, :], main[:, :], mybir.ActivationFunctionType.Sqrt,
                    bias=bias[:, :], scale=2.0**-12, accum_out=dsums[:, :],
                )

            # total = sum_m dsums[m]  (tiny bf16 PE dot)
            total = ps.tile([1, 1], f32, tag="total")
            nc.tensor.matmul(total[:, :], q4, dsums[:, :], start=True, stop=True)
            out_sb = sb.tile([1, 1], f32, tag="out_sb")
            nc.vector.tensor_copy(out_sb[:, :], total[:, :])
            nc.sync.dma_start(out, out_sb[:, :])

    nc.compile()
    return nc


def _get():
    if "nc" not in _CACHE:
        _CACHE["nc"] = _build_nc()
        _CACHE["consts"] = _build_consts()
    return _CACHE["nc"], _CACHE["consts"]


def _in_map(pred, truth):
    import ml_dtypes

    nc, consts = _get()
    p = np.ascontiguousarray(
        np.asarray(pred, dtype=np.float32).reshape(128, 512).astype(ml_dtypes.bfloat16)
    )
    t = np.ascontiguousarray(
        np.asarray(truth, dtype=np.float32).reshape(128, 512).astype(ml_dtypes.bfloat16)
    )
    return nc, {"pred": p, "truth": t, "consts": consts}


def kernel(pred, truth) -> np.ndarray:
    from concourse.bass_utils import run_bass_kernel_spmd

    nc, in_map = _in_map(pred, truth)
    res = run_bass_kernel_spmd(
        nc, [dict(in_map) for _ in range(8)], core_ids=list(range(8))
    )
    return res.results[0]["out"].reshape(()).astype(np.float32)


# revision 15
# speedup vs baseline: 1.1792x; 1.0095x over previous
"""Gcs pairwise-distance loss kernel for Trainium2 (Bass/Tile), 8-core SPMD.

Math: with d = pred - truth, dX = d[:, :P], dY = d[:, P:] (B=32, P=1024),
    sumsq_h[i] = sum_{b,j} (v[b,j] - v[b,i])^2
               = S2_h + sum_b (1024*v[b,i]^2 - 2*rs_h[b]*v[b,i])
where rs_h[b] = sum_j v[b,j], S2_h = sum_{b,j} v[b,j]^2.  The loss is
    (sum_i sqrt(sumsq_X[i]) + sum_i sqrt(sumsq_Y[i])) / 64.
This collapses the O(B*P^2) pairwise reduction to O(B*P).

Layout: d [32, 2048] is viewed as [128, 512]; partition p = 4*b + c where
c in {0,1} covers X columns and {2,3} covers Y columns.

v5 design notes (driven by NTFF traces of v1-v4):
- exec_time is measured first-MEMSET -> last instruction; a fixed ~7us
  NEFF postamble (per-engine semaphore-file zeroing) plus ~2.4us of
  out-DMA receipt + end barriers bound the floor.
- everything runs in bf16 (tolerance 2e-2; measured error ~9e-4).
- PSUM accumulation rule learned in v3: start=True clears the whole
  target BANK's has_written bits, so an accumulation group may not
  have another start TO THE SAME BANK between its start and stop;
  starts to OTHER banks interleave fine.  v5 exploits that:
    hsm2 bank:  pairA(start, red0) ... pairB(stop, red1)
    main bank:  mm1a(start, dsq h0), mm1b(-, dsq h1),
                mm2a(-, W2*td h0), mm2b(stop, W2*td h1)
  (mm1b hits has_written-cleared region b, which writes fresh; the
  mm2 matmuls then accumulate - one group, no same-bank restarts.)
- W2[p,m] = mask01[p,m]*hsm2[p] with the per-partition scalar read
  straight from PSUM (fp32), so the dot term costs one tiny DVE op
  plus PE work that overlaps the DVE/ACT chain.
- per-half row sums on DVE (bf16 tensor_reduce) fill the gap while
  h1 is in flight; per-half ScalarE Squares produce dsq + ssb accums
  concurrently; ssb0+ssb1 merge on DVE feeds one maskS matmul.
- three DMA issue queues: pred halves on sync (HWDGE), truth halves on
  scalar (HWDGE), consts on gpsimd (SWDGE; only needed by ~4.5us): the
  64KB transfers are latency-bound (~2.1us after issue end), so what
  matters is issuing truth h1 as early as possible on a HWDGE queue.
- dsums accumulates in bf16 so the final 4-row dot is a single-pass
  bf16 matmul; gpsimd never needs a custom-op library load.
- a dummy Sqrt right after the DMA issues forces ONE set-3 table load
  (sqrt+square) early, hidden under the DMA flight; without it the
  table pass reloads set 3 right before the final Sqrt.

Every core computes the full replicated result (inputs are only 256KB
in bf16, far below the ~20us collective all-reduce floor, so replication
beats batch-sharding + AllReduce); core 0's scalar is returned.
"""

import numpy as np

_CACHE = {}

H = 288  # column split: h0 = 288 cols, h1 = 224 (the late half is
# gated by DMA arrival + serial sub/reduce, so it gets fewer columns)


def _build_consts():
    # bf16 [128, 141]:
    #   cols 0:128    hconst[p,m]   = -2 if p//2==m//2    (lhsT, pair-sum matmuls)
    #   cols 128:132  mask01[p,m]   = 1 if p%4==m         (base for W2 weights)
    #   cols 132:136  maskS[p,m]    = 2^-12 if (p%4)//2==m//2  (lhsT, S2 matmul)
    #   cols 136:140  mask1024[p,m] = 1024 if p%4==m      (lhsT, d^2 matmuls)
    #   col  140      q4[p]         = 1 if p<4            (lhsT, final dot)
    import ml_dtypes

    c = np.zeros((128, 141), dtype=np.float32)
    p = np.arange(128)
    for m in range(128):
        c[p[p // 2 == m // 2], m] = -2.0
    for m in range(4):
        c[p[p % 4 == m], 128 + m] = 1.0
        c[p[(p % 4) // 2 == m // 2], 132 + m] = 2.0**-12
        c[p[p % 4 == m], 136 + m] = 1024.0
    c[0:4, 140] = 1.0
    return c.astype(ml_dtypes.bfloat16)


def _build_nc():
    import concourse.tile as tile
    from concourse import bacc, mybir

    f32 = mybir.dt.float32
    bf16 = mybir.dt.bfloat16
    Sq = mybir.ActivationFunctionType.Square
    nc = bacc.Bacc("TRN2", target_bir_lowering=False, debug=False)
    pred = nc.dram_tensor("pred", [128, 512], bf16, kind="ExternalInput").ap()
    truth = nc.dram_tensor("truth", [128, 512], bf16, kind="ExternalInput").ap()
    consts = nc.dram_tensor("consts", [128, 141], bf16, kind="ExternalInput").ap()
    out = nc.dram_tensor("out", [1, 1], f32, kind="ExternalOutput").ap()

    with tile.TileContext(nc) as tc:
        with (
            tc.tile_pool(name="sb", bufs=1) as sb,
            tc.tile_pool(name="ps", bufs=1, space="PSUM") as ps,
        ):
            tp0 = sb.tile([128, H], bf16, tag="tp0")
            tt0 = sb.tile([128, H], bf16, tag="tt0")
            tp1 = sb.tile([128, 512 - H], bf16, tag="tp1")
            tt1 = sb.tile([128, 512 - H], bf16, tag="tt1")
            tcst = sb.tile([128, 141], bf16, tag="tcst")
            nc.sync.dma_start(tp0[:, :], pred[:, 0:H])
            nc.scalar.dma_start(tt0[:, :], truth[:, 0:H])
            nc.gpsimd.dma_start(tcst[:, :], consts)
            nc.sync.dma_start(tp1[:, :], pred[:, H:512])
            nc.scalar.dma_start(tt1[:, :], truth[:, H:512])
            hconst = tcst[:, 0:128]
            mask01 = tcst[:, 128:132]
            maskS = tcst[:, 132:136]
            mask1024 = tcst[:, 136:140]
            q4 = tcst[0:4, 140:141]

            # Dummy Sqrt on a bass preamble const: forces the ACT table
            # pass to load set 3 (sqrt+square) early, hidden under the
            # DMA flight, and to keep it for the Squares AND final Sqrt.
            # Without it the pass picks set 0 for the Squares and reloads
            # set 3 right before the final Sqrt (1.28us on the critical
            # path, gated behind the bias wait).
            dummy = sb.tile([128, 1], f32, tag="dummy")
            nc.scalar.activation(
                dummy[:, :],
                nc.const_aps.aps[(f32, 1.0)],
                mybir.ActivationFunctionType.Sqrt,
            )

            td = sb.tile([128, 512], bf16, tag="td")
            red0 = sb.tile([128, 1], bf16, tag="red0")
            red1 = sb.tile([128, 1], bf16, tag="red1")
            dsq = sb.tile([128, 512], bf16, tag="dsq")
            ssb0 = sb.tile([128, 1], bf16, tag="ssb0")
            ssb1 = sb.tile([128, 1], bf16, tag="ssb1")

            # h0: sub + row sums (DVE) and squares + ssb accum (ACT)
            # run in the gap while h1 is still in flight.
            nc.vector.tensor_sub(td[:, 0:H], tp0[:, :], tt0[:, :])
            with nc.allow_low_precision("row sums feed the ~0.5% dot term"):
                nc.vector.tensor_reduce(
                    out=red0[:, :], in_=td[:, 0:H], axis=mybir.AxisListType.X,
                    op=mybir.AluOpType.add,
                )
            with nc.allow_low_precision("ssb: S2 sums in bf16, ~0.4% noise"):
                nc.scalar.activation(
                    dsq[:, 0:H], td[:, 0:H], Sq, accum_out=ssb0[:, :],
                )
            # h1
            nc.vector.tensor_sub(td[:, H:512], tp1[:, :], tt1[:, :])
            with nc.allow_low_precision("row sums feed the ~0.5% dot term"):
                nc.vector.tensor_reduce(
                    out=red1[:, :], in_=td[:, H:512], axis=mybir.AxisListType.X,
                    op=mybir.AluOpType.add,
                )
            with nc.allow_low_precision("ssb: S2 sums in bf16, ~0.4% noise"):
                nc.scalar.activation(
                    dsq[:, H:512], td[:, H:512], Sq, accum_out=ssb1[:, :],
                )

            # hsm2[p] = -2*(r[p]+r[p^1]): accumulate red0 then red1 in the
            # hsm2 bank; main bank: one group, one start (mm1a), regions
            # written fresh after the bank-wide has_written clear, then
            # the two W2 matmuls accumulate the dot term on top.
            hsm2 = ps.tile([128, 1], f32, tag="hsm2")
            main = ps.tile([4, 512], f32, tag="main")
            s2 = ps.tile([4, 1], f32, tag="s2")
            nc.tensor.matmul(hsm2[:, :], hconst, red0[:, :], start=True, stop=False)
            nc.tensor.matmul(main[:, 0:H], mask1024, dsq[:, 0:H], start=True, stop=False)
            nc.tensor.matmul(hsm2[:, :], hconst, red1[:, :], start=False, stop=True)
            nc.tensor.matmul(
                main[:, H:512], mask1024, dsq[:, H:512], start=False, stop=False,
                skip_group_check=True,
            )

            # W2[p,m] = mask01[p,m] * hsm2[p], scalar read from PSUM
            w2 = sb.tile([128, 4], bf16, tag="w2")
            nc.vector.tensor_scalar(
                w2[:, :], mask01, hsm2[:, :], None, mybir.AluOpType.mult,
            )
            # ssb = ssb0 + ssb1 feeds one maskS matmul
            ssb = sb.tile([128, 1], bf16, tag="ssb")
            nc.vector.tensor_add(ssb[:, :], ssb0[:, :], ssb1[:, :])

            nc.tensor.matmul(
                main[:, 0:H], w2[:, :], td[:, 0:H], start=False, stop=False,
                skip_group_check=True,
            )
            nc.tensor.matmul(s2[:, :], maskS, ssb[:, :], start=True, stop=True)
            nc.tensor.matmul(
                main[:, H:512], w2[:, :], td[:, H:512], start=False, stop=True,
                skip_group_check=True,
            )

            bias = sb.tile([4, 1], f32, tag="bias")
            nc.vector.tensor_copy(bias[:, :], s2[:, :])

            # dist = sqrt(main*2^-12 + bias) = sqrt(sumsq)/64; dsums[m]
            # accumulates in bf16 so the final dot is single-pass bf16.
            dist = sb.tile([4, 512], bf16, tag="dist")
            dsums = sb.tile([4, 1], bf16, tag="dsums")
            with nc.allow_low_precision("dsums ~2900, bf16 rounding ~0.03%"):
                nc.scalar.activation(
                    dist[:, :], main[:, :], mybir.ActivationFunctionType.Sqrt,
                    bias=bias[:, :], scale=2.0**-12, accum_out=dsums[:, :],
                )

            # total = sum_m dsums[m]  (tiny bf16 PE dot)
            total = ps.tile([1, 1], f32, tag="total")
            nc.tensor.matmul(total[:, :], q4, dsums[:, :], start=True, stop=True)
            out_sb = sb.tile([1, 1], f32, tag="out_sb")
            nc.vector.tensor_copy(out_sb[:, :], total[:, :])
            nc.sync.dma_start(out, out_sb[:, :], single_packet=True)

    nc.compile()
    return nc


def _get():
    if "nc" not in _CACHE:
        _CACHE["nc"] = _build_nc()
        _CACHE["consts"] = _build_consts()
    return _CACHE["nc"], _CACHE["consts"]


def _in_map(pred, truth):
    import ml_dtypes

    nc, consts = _get()
    p = np.ascontiguousarray(
        np.asarray(pred, dtype=np.float32).reshape(128, 512).astype(ml_dtypes.bfloat16)
    )
    t = np.ascontiguousarray(
        np.asarray(truth, dtype=np.float32).reshape(128, 512).astype(ml_dtypes.bfloat16)
    )
    return nc, {"pred": p, "truth": t, "consts": consts}


def kernel(pred, truth) -> np.ndarray:
    from concourse.bass_utils import run_bass_kernel_spmd

    nc, in_map = _in_map(pred, truth)
    res = run_bass_kernel_spmd(
        nc, [dict(in_map) for _ in range(8)], core_ids=list(range(8))
    )
    return res.results[0]["out"].reshape(()).astype(np.float32)
